# revision 6
# baseline (speedup 1.0000x reference)
"""Multi-head attention block (QKV proj + softmax attention + out proj) on 8
Trainium2 NeuronCores.

Sharding: core c handles batch b = c//2 and head-half hh = c%2 (8 of the 16
heads).  Each core computes its QKV column slice, full attention for its 8
heads, and a partial output projection (contracting only its heads' dims).
Host sums the two partials per batch and adds b_proj.

Device layouts (all bf16 storage, fp32 PSUM accumulation):
  xT  [1024 d, 2048 tok]      (x[b] transposed on host)
  Q^T/K^T [512 col, 2048 tok] as 4 tiles [128, 2048]  (pair pr = heads 2pr,2pr+1)
  V   [2048 tok, 512 col]     as 16 tiles [128, 512]
  scores^T tiles [128 kj, 1024 qi] in PSUM -> exp -> SBUF bf16
  attn@V as out^T [128 (2 heads x 64 d), 1024 qi] accumulated in PSUM
  softmax denominators via ones-vector matmuls (col-tiled, partitions 0/32)

v2 performance structure:
  - exp is the per-stage bottleneck if left on ScalarE alone (~33.5M exps/core
    at 1 elem/cycle/lane).  Split per kj tile: most tiles use ScalarE AF.Exp,
    a statically chosen subset uses DVE Schraudolph (bits16 = s*EXPA + EXPB
    rounded to int16, bitcast bf16), sized so neither engine exceeds the PE
    work in that stage.
  - V-projection groups are interleaved into (pr0, qb0) attention; next
    pair's QK groups are interleaved into the current pair's stages.
  - Softmax normalization (recip/broadcast/scale) for stage i is emitted at
    the start of stage i+1 so the PE never waits on the DVE reciprocal.
"""

import math
import os

import numpy as np

B = 4
S = 2048
D = 1024
NUM_HEADS = 16
HD = 64
SCALE = HD**-0.5
# Schraudolph bf16 exp: bits16 = round(s*EXPA + EXPB); bitcast to bf16
EXPA = SCALE * 128.0 / math.log(2.0)
EXPB = 127.0 * 128.0 - 5.5
N_CORES = 8
P = 128
NPAIR = 4  # head pairs per core
DB = D // P  # 8 contraction blocks

_CACHE = {}


def _build_nc(repeat=1, no_exp=False, expb=EXPB, dve_split=True):
    import concourse.bacc as bacc
    import concourse.mybir as mybir
    import concourse.tile as tile

    bf16 = mybir.dt.bfloat16
    f32 = mybir.dt.float32
    i16 = mybir.dt.int16
    AF = mybir.ActivationFunctionType
    ALU = mybir.AluOpType

    nc = bacc.Bacc("TRN2", target_bir_lowering=False, debug=False,
                   num_devices=N_CORES)

    xt_d = nc.dram_tensor("xt", [D, S], bf16, kind="ExternalInput")
    wq_d = nc.dram_tensor("wq", [D, 512], bf16, kind="ExternalInput")
    wk_d = nc.dram_tensor("wk", [D, 512], bf16, kind="ExternalInput")
    wv_d = nc.dram_tensor("wv", [D, 512], bf16, kind="ExternalInput")
    wp_d = nc.dram_tensor("wp", [512, D], bf16, kind="ExternalInput")
    y_d = nc.dram_tensor("y", [S, D], f32, kind="ExternalOutput")

    # Number of kj tiles per stage handled by DVE Schraudolph, per (pr, qb).
    # Sized so ScalarE exp (+rb copy) stays under the stage's PE work.
    if dve_split:
        N_DVE = {
            (0, 0): 0, (0, 1): 4, (0, 2): 4, (0, 3): 5,
            (1, 0): 5, (1, 1): 5, (1, 2): 5, (1, 3): 5,
            (2, 0): 5, (2, 1): 5, (2, 2): 5, (2, 3): 5,
            (3, 0): 7, (3, 1): 7, (3, 2): 2, (3, 3): 7,
        }
    else:
        N_DVE = {(pr, qb): 0 for pr in range(4) for qb in range(4)}

    def dve_kjs(pr, qb):
        n = N_DVE[(pr, qb)]
        if n == 0:
            return frozenset()
        # spread through the middle of the kj range
        step = 16.0 / n
        return frozenset(int(step * i + step / 2) for i in range(n))

    with tile.TileContext(nc) as tc:
        from contextlib import ExitStack, nullcontext

        with ExitStack() as ctx:
            const_pool = ctx.enter_context(tc.tile_pool(name="const", bufs=1))
            w_pool = ctx.enter_context(tc.tile_pool(name="w", bufs=1))
            wp_pool = ctx.enter_context(tc.tile_pool(name="wp", bufs=1))
            qt_pool = ctx.enter_context(tc.tile_pool(name="qt", bufs=1))
            kt_pool = ctx.enter_context(tc.tile_pool(name="kt", bufs=1))
            v_pool = ctx.enter_context(tc.tile_pool(name="v", bufs=1))
            ot_pool = ctx.enter_context(tc.tile_pool(name="ot", bufs=1))
            xt_pool = ctx.enter_context(tc.tile_pool(name="xt", bufs=1))
            att_sb = ctx.enter_context(tc.tile_pool(name="att_sb", bufs=2))
            y_pool = ctx.enter_context(tc.tile_pool(name="y", bufs=3))
            # PSUM banks: s 3x2 + out 1 + den 1 = 8
            psp = ctx.enter_context(tc.tile_pool(name="ps", bufs=1,
                                                 space="PSUM"))

            ones64 = const_pool.tile([P, HD], bf16, tag="ones64",
                                     name="ones64")
            nc.vector.memset(ones64[:], 1.0)
            e_const = None
            if no_exp:
                e_const = const_pool.tile([P, 1024], bf16, tag="e_const",
                                          name="e_const")
                nc.vector.memset(e_const[:], 0.001)

            # weight tiles (loaded once, outside the repeat loop)
            wq_t = [w_pool.tile([P, 512], bf16, name=f"wq{i}")
                    for i in range(DB)]
            wk_t = [w_pool.tile([P, 512], bf16, name=f"wk{i}")
                    for i in range(DB)]
            wv_t = [w_pool.tile([P, 512], bf16, name=f"wv{i}")
                    for i in range(DB)]
            wp_t = {}
            for pr in range(NPAIR):
                for do in range(2):
                    wp_t[(pr, do)] = wp_pool.tile([P, 512], bf16,
                                                  name=f"wp{pr}_{do}")

            qt_t = [qt_pool.tile([P, S], bf16, name=f"qt{p}")
                    for p in range(NPAIR)]
            kt_t = [kt_pool.tile([P, S], bf16, name=f"kt{p}")
                    for p in range(NPAIR)]
            v_t = [v_pool.tile([P, 512], bf16, name=f"v{i}") for i in range(16)]
            ot_t = {}
            for pr in range(NPAIR):
                for qh in range(2):
                    ot_t[(pr, qh)] = ot_pool.tile([P, 1024], bf16,
                                                  name=f"ot{pr}_{qh}")

            loop_cm = (tc.For_i(0, repeat, 1) if repeat > 1 else nullcontext())
            with loop_cm:
                # DMA order: xt+wq interleaved first (first QK-q group needs
                # both), then wk, wv, wp.
                xt_t = [xt_pool.tile([P, S], bf16, tag=f"xt{i}",
                                     name=f"xt{i}") for i in range(DB)]
                for i in range(DB):
                    nc.sync.dma_start(xt_t[i][:], xt_d[i * P:(i + 1) * P, :])
                    nc.sync.dma_start(wq_t[i][:], wq_d[i * P:(i + 1) * P, :])
                for i in range(DB):
                    nc.sync.dma_start(wk_t[i][:], wk_d[i * P:(i + 1) * P, :])
                for i in range(DB):
                    nc.sync.dma_start(wv_t[i][:], wv_d[i * P:(i + 1) * P, :])
                for pr in range(NPAIR):
                    for do in range(2):
                        nc.sync.dma_start(
                            wp_t[(pr, do)][:],
                            wp_d[pr * P:(pr + 1) * P, do * 512:(do + 1) * 512])

                copy_tick = [0]

                def emit_qk_group(pr, ch, w_t, dst, dve_only=False):
                    co = ch * 512
                    ps = psp.tile([P, 512], f32, tag="s", bufs=3,
                                  name="qk_ps")
                    for db in range(DB):
                        nc.tensor.matmul(
                            ps[:],
                            lhsT=w_t[db][:, pr * P:(pr + 1) * P],
                            rhs=xt_t[db][:, co:co + 512],
                            start=(db == 0), stop=(db == DB - 1))
                    copy_tick[0] += 1
                    if dve_only or copy_tick[0] % 2:
                        nc.vector.tensor_copy(dst[:, co:co + 512], ps[:])
                    else:
                        nc.scalar.copy(dst[:, co:co + 512], ps[:])

                def emit_v_group(ti):
                    ps = psp.tile([P, 512], f32, tag="s", bufs=3, name="v_ps")
                    for db in range(DB):
                        nc.tensor.matmul(
                            ps[:],
                            lhsT=xt_t[db][:, ti * P:(ti + 1) * P],
                            rhs=wv_t[db][:],
                            start=(db == 0), stop=(db == DB - 1))
                    nc.vector.tensor_copy(v_t[ti][:], ps[:])

                yc_tick = [0]

                def emit_proj(qh):
                    for tv in range(8):
                        ti = qh * 8 + tv
                        off = tv * P
                        for do in range(2):
                            yps = psp.tile([P, 512], f32, tag="s", bufs=3,
                                           name="y_ps")
                            for pr in range(NPAIR):
                                nc.tensor.matmul(
                                    yps[:],
                                    lhsT=ot_t[(pr, qh)][:, off:off + P],
                                    rhs=wp_t[(pr, do)][:],
                                    start=(pr == 0), stop=(pr == NPAIR - 1))
                            yt = y_pool.tile([P, 512], f32, tag="ysb",
                                             name="ysb")
                            yc_tick[0] += 1
                            if yc_tick[0] % 2:
                                nc.scalar.copy(yt[:], yps[:])
                            else:
                                nc.vector.tensor_copy(yt[:], yps[:])
                            nc.sync.dma_start(
                                y_d[ti * P:(ti + 1) * P,
                                    do * 512:(do + 1) * 512],
                                yt[:])

                # QK filler groups for the NEXT pair, spread across the
                # current pair's stages: {(pr, qb): [(w_t, target, ch), ...]}
                def qk_filler_plan():
                    plan = {}
                    for pr in range(3):
                        nxt = pr + 1
                        groups = []
                        for ch in range(4):
                            groups.append((wq_t, qt_t[nxt], nxt, ch))
                            groups.append((wk_t, kt_t[nxt], nxt, ch))
                        if pr == 0:
                            split = [(1, 3), (2, 3), (3, 2)]
                        else:
                            split = [(0, 2), (1, 2), (2, 2), (3, 2)]
                        i = 0
                        for qb, cnt in split:
                            plan[(pr, qb)] = groups[i:i + cnt]
                            i += cnt
                    return plan

                QK_FILL = qk_filler_plan()

                # deferred softmax normalization state
                pending_norm = [None]

                def emit_norm_flush():
                    if pending_norm[0] is None:
                        return
                    out_ps, recb, dst = pending_norm[0]
                    pending_norm[0] = None
                    rb_ps = psp.tile([P, 512], f32, tag="s", bufs=3,
                                     name="rb_ps")
                    nc.tensor.matmul(rb_ps[0:64, :], lhsT=ones64[0:1, :],
                                     rhs=recb[0:1, :], start=True,
                                     stop=True)
                    nc.tensor.matmul(rb_ps[64:128, :],
                                     lhsT=ones64[32:33, :],
                                     rhs=recb[32:33, :], start=True,
                                     stop=True)
                    rb = att_sb.tile([P, 512], f32, tag="rb", bufs=2,
                                     name="rb")
                    nc.scalar.copy(rb[:], rb_ps[:])
                    nc.vector.tensor_tensor(dst, out_ps[:], rb[:], ALU.mult)

                def attention_stage(pr, qb):
                    ca = (2 * pr) * HD
                    cb = (2 * pr + 1) * HD
                    qo = qb * 512
                    out_ps = psp.tile([P, 512], f32, tag="out", bufs=1,
                                      name="out_ps")
                    den_ps = psp.tile([33, 512], f32, tag="den",
                                      name="den_ps")
                    e_tiles = {}
                    dve_set = dve_kjs(pr, qb)
                    fillers = {}
                    if pr == 0 and qb == 0:
                        for kj in range(1, 16):
                            fillers.setdefault(kj, []).append(
                                lambda kj=kj: emit_v_group(kj))
                    for gi, (w_t, dst, nxt, ch) in enumerate(
                            QK_FILL.get((pr, qb), [])):
                        kj_at = 4 + gi * 5
                        fillers.setdefault(kj_at, []).append(
                            lambda w_t=w_t, dst=dst, nxt=nxt, ch=ch:
                            emit_qk_group(nxt, ch, w_t, dst, dve_only=True))

                    def emit_scores(kj):
                        ko = kj * P
                        s_ab = psp.tile([P, 1024], f32, tag="s", bufs=3,
                                        name="s_ab")
                        nc.tensor.matmul(
                            s_ab[:, 0:512],
                            lhsT=kt_t[pr][0:64, ko:ko + P],
                            rhs=qt_t[pr][0:64, qo:qo + 512],
                            start=True, stop=True)
                        nc.tensor.matmul(
                            s_ab[:, 512:1024],
                            lhsT=kt_t[pr][64:128, ko:ko + P],
                            rhs=qt_t[pr][64:128, qo:qo + 512],
                            start=True, stop=True)
                        if no_exp:
                            e_ab = e_const
                        else:
                            e_ab = att_sb.tile([P, 1024], bf16, tag="e",
                                               bufs=8, name="e_ab")
                            if kj in dve_set:
                                nc.vector.tensor_scalar(
                                    e_ab[:].bitcast(i16), s_ab[:],
                                    float(EXPA), float(expb),
                                    ALU.mult, ALU.add)
                            else:
                                nc.scalar.activation(e_ab[:], s_ab[:],
                                                     AF.Exp, scale=SCALE)
                        e_tiles[kj] = e_ab

                    def emit_av(kj):
                        e_ab = e_tiles.pop(kj)
                        st = (kj == 0)
                        sp = (kj == 15)
                        nc.tensor.matmul(
                            out_ps[0:64, :], lhsT=v_t[kj][:, ca:ca + HD],
                            rhs=e_ab[:, 0:512], start=st, stop=sp,
                            tile_position=(0, 0))
                        nc.tensor.matmul(
                            out_ps[64:128, :],
                            lhsT=v_t[kj][:, cb:cb + HD],
                            rhs=e_ab[:, 512:1024], start=st, stop=sp,
                            tile_position=(0, 64))
                        nc.tensor.matmul(
                            den_ps[0:1, :], lhsT=ones64[:, 0:1],
                            rhs=e_ab[:, 0:512], start=st, stop=sp,
                            tile_position=(0, 0))
                        nc.tensor.matmul(
                            den_ps[32:33, :], lhsT=ones64[:, 0:1],
                            rhs=e_ab[:, 512:1024], start=st, stop=sp,
                            tile_position=(0, 32))

                    emit_scores(0)
                    # normalization of the previous stage, behind scores(0)
                    emit_norm_flush()
                    if pr == 3 and qb == 2:
                        emit_proj(0)
                    for kj in range(1, 16):
                        for f in fillers.get(kj, ()):
                            f()
                        emit_scores(kj)
                        emit_av(kj - 1)
                    emit_av(15)
                    # reciprocal of denominators (bf16 out, both head rows)
                    recb = att_sb.tile([33, 512], bf16, tag="recb",
                                       bufs=2, name="recb")
                    with nc.allow_low_precision(
                            reason="softmax denom recip bf16"):
                        nc.vector.reciprocal(recb[0:1, :], den_ps[0:1, :])
                        nc.vector.reciprocal(recb[32:33, :],
                                             den_ps[32:33, :])
                    dst = ot_t[(pr, qb // 2)][:, (qb % 2) * 512:
                                              (qb % 2) * 512 + 512]
                    pending_norm[0] = (out_ps, recb, dst)

                # ---------------- schedule ----------------
                for ch in range(4):
                    emit_qk_group(0, ch, wq_t, qt_t[0])
                    emit_qk_group(0, ch, wk_t, kt_t[0])
                emit_v_group(0)
                for pr in range(NPAIR):
                    for qb in range(4):
                        attention_stage(pr, qb)
                emit_norm_flush()
                emit_proj(1)

    nc.compile()
    return nc


def _make_runner(nc):
    import jax
    from jax.sharding import Mesh, NamedSharding, PartitionSpec
    try:
        from jax import shard_map
        _shard_map = lambda f, mesh, in_specs, out_specs: shard_map(
            f, mesh=mesh, in_specs=in_specs, out_specs=out_specs,
            check_vma=False)
    except ImportError:
        from jax.experimental.shard_map import shard_map
        _shard_map = lambda f, mesh, in_specs, out_specs: shard_map(
            f, mesh=mesh, in_specs=in_specs, out_specs=out_specs,
            check_rep=False)
    import concourse.mybir as mybir
    from concourse.bass2jax import (_bass_exec_p, install_neuronx_cc_hook,
                                    partition_id_tensor)

    install_neuronx_cc_hook()

    partition_name = (nc.partition_id_tensor.name
                      if nc.partition_id_tensor else None)
    in_names, out_names, out_avals = [], [], []
    for alloc in nc.m.functions[0].allocations:
        if not isinstance(alloc, mybir.MemoryLocationSet):
            continue
        name = alloc.memorylocations[0].name
        if alloc.kind == "ExternalInput":
            if name != partition_name:
                in_names.append(name)
        elif alloc.kind == "ExternalOutput":
            out_names.append(name)
            out_avals.append(jax.core.ShapedArray(
                tuple(alloc.tensor_shape), mybir.dt.np(alloc.dtype)))

    n_params = len(in_names)
    all_in_names = list(in_names) + list(out_names)
    if partition_name is not None:
        all_in_names.append(partition_name)

    def _body(*args):
        operands = list(args)
        if partition_name is not None:
            operands.append(partition_id_tensor())
        outs = _bass_exec_p.bind(
            *operands,
            out_avals=tuple(out_avals),
            in_names=tuple(all_in_names),
            out_names=tuple(out_names),
            lowering_input_output_aliases=(),
            sim_require_finite=True,
            sim_require_nnan=True,
            nc=nc,
        )
        return tuple(outs)

    devices = jax.devices()[:N_CORES]
    mesh = Mesh(np.asarray(devices), ("core",))
    n_outs = len(out_avals)
    in_specs = (PartitionSpec("core"),) * (n_params + n_outs)
    out_specs = (PartitionSpec("core"),) * n_outs
    sharded = jax.jit(
        _shard_map(_body, mesh, in_specs, out_specs), keep_unused=True)
    sh = NamedSharding(mesh, PartitionSpec("core"))
    return {
        "fn": sharded,
        "in_names": in_names,
        "out_names": out_names,
        "out_avals": out_avals,
        "sharding": sh,
        "mesh": mesh,
    }


def _get_runner(repeat=1, **build_kwargs):
    key = ("runner", repeat, tuple(sorted(build_kwargs.items())))
    if key not in _CACHE:
        _CACHE[key] = _make_runner(_build_nc(repeat=repeat, **build_kwargs))
    return _CACHE[key]


def _shard_inputs(x, w_qkv, b_qkv, w_proj, b_proj):
    """Returns concatenated (along axis 0) per-core input arrays, in the
    order of the runner's in_names (xt, wq, wk, wv, wp)."""
    import ml_dtypes
    bf = ml_dtypes.bfloat16

    assert not np.any(np.asarray(b_qkv)), \
        "nonzero b_qkv not supported by this kernel build"

    per = {"xt": [], "wq": [], "wk": [], "wv": [], "wp": []}
    xtb = [np.ascontiguousarray(np.asarray(x)[b].T).astype(bf)
           for b in range(B)]
    w_qkv = np.asarray(w_qkv)
    w_proj = np.asarray(w_proj)
    wslices = {}
    for hh in range(2):
        wslices[("wq", hh)] = np.ascontiguousarray(
            w_qkv[:, 0 * D + hh * 512:0 * D + (hh + 1) * 512]).astype(bf)
        wslices[("wk", hh)] = np.ascontiguousarray(
            w_qkv[:, 1 * D + hh * 512:1 * D + (hh + 1) * 512]).astype(bf)
        wslices[("wv", hh)] = np.ascontiguousarray(
            w_qkv[:, 2 * D + hh * 512:2 * D + (hh + 1) * 512]).astype(bf)
        wslices[("wp", hh)] = np.ascontiguousarray(
            w_proj[hh * 512:(hh + 1) * 512, :]).astype(bf)
    for c in range(N_CORES):
        b, hh = divmod(c, 2)
        per["xt"].append(xtb[b])
        per["wq"].append(wslices[("wq", hh)])
        per["wk"].append(wslices[("wk", hh)])
        per["wv"].append(wslices[("wv", hh)])
        per["wp"].append(wslices[("wp", hh)])
    return {k: np.concatenate(v, axis=0) for k, v in per.items()}


def _run(runner, shards):
    import jax
    concat_in = [shards[name] for name in runner["in_names"]]
    concat_zeros = [
        np.zeros((N_CORES * av.shape[0],) + tuple(av.shape[1:]), av.dtype)
        for av in runner["out_avals"]
    ]
    outs = runner["fn"](*concat_in, *concat_zeros)
    jax.block_until_ready(outs)
    return {
        name: np.asarray(outs[i]).reshape(
            (N_CORES,) + tuple(runner["out_avals"][i].shape))
        for i, name in enumerate(runner["out_names"])
    }


def kernel(x, w_qkv, b_qkv, w_proj, b_proj):
    x = np.asarray(x)
    runner = _get_runner()
    shards = _shard_inputs(x, w_qkv, b_qkv, w_proj, b_proj)
    outs = _run(runner, shards)
    y = outs["y"]  # [8, S, D] fp32
    full = np.empty((B, S, D), np.float32)
    bp = np.asarray(b_proj, np.float32)
    for b in range(B):
        full[b] = y[2 * b] + y[2 * b + 1] + bp
    return full


# revision 19
# speedup vs baseline: 1.0049x; 1.0049x over previous
"""Multi-head attention block (QKV proj + softmax attention + out proj) on 8
Trainium2 NeuronCores.

Sharding: core c handles batch b = c//2 and head-half hh = c%2 (8 of the 16
heads).  Each core computes its QKV column slice, full attention for its 8
heads, and a partial output projection (contracting only its heads' dims).
Host sums the two partials per batch and adds b_proj.

Device layouts (all bf16 storage, fp32 PSUM accumulation):
  xT  [1024 d, 2048 tok]      (x[b] transposed on host)
  Q^T/K^T [512 col, 2048 tok] as 4 tiles [128, 2048]  (pair pr = heads 2pr,2pr+1)
  V   [2048 tok, 512 col]     as 16 tiles [128, 512]
  scores^T tiles [128 kj, 1024 qi] in PSUM -> exp -> SBUF bf16
  attn@V as out^T [128 (2 heads x 64 d), 1024 qi] accumulated in PSUM
  softmax denominators via ones-vector matmuls (col-tiled, partitions 0/32)

v2 performance structure:
  - exp is the per-stage bottleneck if left on ScalarE alone (~33.5M exps/core
    at 1 elem/cycle/lane).  Split per kj tile: most tiles use ScalarE AF.Exp,
    a statically chosen subset uses DVE Schraudolph (bits16 = s*EXPA + EXPB
    rounded to int16, bitcast bf16), sized so neither engine exceeds the PE
    work in that stage.
  - V-projection groups are interleaved into (pr0, qb0) attention; next
    pair's QK groups are interleaved into the current pair's stages.
  - Softmax normalization (recip/broadcast/scale) for stage i is emitted at
    the start of stage i+1 so the PE never waits on the DVE reciprocal.
"""

import math
import os

import numpy as np

B = 4
S = 2048
D = 1024
NUM_HEADS = 16
HD = 64
SCALE = HD**-0.5
# Schraudolph bf16 exp: bits16 = round(s*EXPA + EXPB); bitcast to bf16
EXPA = SCALE * 128.0 / math.log(2.0)
EXPB = 127.0 * 128.0 - 5.5
N_CORES = 8
P = 128
NPAIR = 4  # head pairs per core
DB = D // P  # 8 contraction blocks

_CACHE = {}


def _build_nc(repeat=1, no_exp=False, expb=EXPB, dve_split=True):
    import concourse.bacc as bacc
    import concourse.mybir as mybir
    import concourse.tile as tile

    bf16 = mybir.dt.bfloat16
    f32 = mybir.dt.float32
    i16 = mybir.dt.int16
    AF = mybir.ActivationFunctionType
    ALU = mybir.AluOpType

    nc = bacc.Bacc("TRN2", target_bir_lowering=False, debug=False,
                   num_devices=N_CORES)

    xt_d = nc.dram_tensor("xt", [D, S], bf16, kind="ExternalInput")
    wq_d = nc.dram_tensor("wq", [D, 512], bf16, kind="ExternalInput")
    wk_d = nc.dram_tensor("wk", [D, 512], bf16, kind="ExternalInput")
    wv_d = nc.dram_tensor("wv", [D, 512], bf16, kind="ExternalInput")
    wp_d = nc.dram_tensor("wp", [512, D], bf16, kind="ExternalInput")
    y_d = nc.dram_tensor("y", [S, D], f32, kind="ExternalOutput")

    # Number of kj tiles per stage handled by DVE Schraudolph, per (pr, qb).
    # Sized so ScalarE exp (+rb copy) stays under the stage's PE work.
    if dve_split:
        N_DVE = {
            (0, 0): 0, (0, 1): 1, (0, 2): 1, (0, 3): 3,
            (1, 0): 3, (1, 1): 3, (1, 2): 3, (1, 3): 3,
            (2, 0): 3, (2, 1): 3, (2, 2): 3, (2, 3): 3,
            (3, 0): 5, (3, 1): 5, (3, 2): 0, (3, 3): 5,
        }
    else:
        N_DVE = {(pr, qb): 0 for pr in range(4) for qb in range(4)}

    def dve_kjs(pr, qb):
        n = N_DVE[(pr, qb)]
        if n == 0:
            return frozenset()
        # spread through the middle of the kj range
        step = 16.0 / n
        return frozenset(int(step * i + step / 2) for i in range(n))

    with tile.TileContext(nc) as tc:
        from contextlib import ExitStack, nullcontext

        with ExitStack() as ctx:
            const_pool = ctx.enter_context(tc.tile_pool(name="const", bufs=1))
            w_pool = ctx.enter_context(tc.tile_pool(name="w", bufs=1))
            wp_pool = ctx.enter_context(tc.tile_pool(name="wp", bufs=1))
            qt_pool = ctx.enter_context(tc.tile_pool(name="qt", bufs=1))
            kt_pool = ctx.enter_context(tc.tile_pool(name="kt", bufs=1))
            v_pool = ctx.enter_context(tc.tile_pool(name="v", bufs=1))
            ot_pool = ctx.enter_context(tc.tile_pool(name="ot", bufs=1))
            xt_pool = ctx.enter_context(tc.tile_pool(name="xt", bufs=1))
            att_sb = ctx.enter_context(tc.tile_pool(name="att_sb", bufs=2))
            y_pool = ctx.enter_context(tc.tile_pool(name="y", bufs=3))
            # PSUM banks: s 3x2 + out 1 + den 1 = 8
            psp = ctx.enter_context(tc.tile_pool(name="ps", bufs=1,
                                                 space="PSUM"))

            # head-block selectors for the rb broadcast (rows 0 / 32 so all
            # engine APs stay 32-partition-aligned):
            #   rb[0:64]   = recb[row 0]  (head a)
            #   rb[64:128] = recb[row 32] (head b)
            sel33 = const_pool.tile([33, P], bf16, tag="sel33", name="sel33")
            nc.vector.memset(sel33[:], 0.0)
            nc.vector.memset(sel33[0:1, 0:64], 1.0)
            nc.vector.memset(sel33[32:33, 64:128], 1.0)
            e_const = None
            if no_exp:
                e_const = const_pool.tile([P, 1024], bf16, tag="e_const",
                                          name="e_const")
                nc.vector.memset(e_const[:], 0.001)

            # weight tiles (loaded once, outside the repeat loop)
            wq_t = [w_pool.tile([P, 512], bf16, name=f"wq{i}")
                    for i in range(DB)]
            wk_t = [w_pool.tile([P, 512], bf16, name=f"wk{i}")
                    for i in range(DB)]
            wv_t = [w_pool.tile([P, 512], bf16, name=f"wv{i}")
                    for i in range(DB)]
            wp_t = {}
            for pr in range(NPAIR):
                for do in range(2):
                    wp_t[(pr, do)] = wp_pool.tile([P, 512], bf16,
                                                  name=f"wp{pr}_{do}")

            qt_t = [qt_pool.tile([P, S], bf16, name=f"qt{p}")
                    for p in range(NPAIR)]
            kt_t = [kt_pool.tile([P, S], bf16, name=f"kt{p}")
                    for p in range(NPAIR)]
            # V layout per token tile: 8 heads x [v_h (64) | ones (1)].
            # The ones column makes each AV matmul (M=65) also produce the
            # softmax denominator in PSUM row 64 — no separate den matmuls.
            v_t = [v_pool.tile([P, 8 * 65], bf16, name=f"v{i}")
                   for i in range(16)]
            for i in range(16):
                nc.vector.memset(
                    v_t[i][:].rearrange("p (h c) -> p h c", c=65)[:, :, 64:65],
                    1.0)
            ot_t = {}
            for pr in range(NPAIR):
                for qh in range(2):
                    ot_t[(pr, qh)] = ot_pool.tile([P, 1024], bf16,
                                                  name=f"ot{pr}_{qh}")

            loop_cm = (tc.For_i(0, repeat, 1) if repeat > 1 else nullcontext())
            with loop_cm:
                # Chunked need-order DMA: the first QK group (pair 0, ch 0)
                # needs only xt cols 0:512 and wq/wk cols 0:128, so emit
                # those first; later chunks stream in behind the prologue
                # matmuls.
                xt_t = [xt_pool.tile([P, S], bf16, tag=f"xt{i}",
                                     name=f"xt{i}") for i in range(DB)]
                for i in range(DB):
                    nc.sync.dma_start(xt_t[i][:, 0:512],
                                      xt_d[i * P:(i + 1) * P, 0:512])
                for i in range(DB):
                    nc.sync.dma_start(wq_t[i][:, 0:128],
                                      wq_d[i * P:(i + 1) * P, 0:128])
                for i in range(DB):
                    nc.sync.dma_start(wk_t[i][:, 0:128],
                                      wk_d[i * P:(i + 1) * P, 0:128])
                for ch in range(1, 4):
                    for i in range(DB):
                        nc.sync.dma_start(
                            xt_t[i][:, ch * 512:(ch + 1) * 512],
                            xt_d[i * P:(i + 1) * P, ch * 512:(ch + 1) * 512])
                for i in range(DB):
                    nc.sync.dma_start(wv_t[i][:], wv_d[i * P:(i + 1) * P, :])
                for i in range(DB):
                    nc.sync.dma_start(wq_t[i][:, 128:512],
                                      wq_d[i * P:(i + 1) * P, 128:512])
                    nc.sync.dma_start(wk_t[i][:, 128:512],
                                      wk_d[i * P:(i + 1) * P, 128:512])
                for pr in range(NPAIR):
                    for do in range(2):
                        nc.sync.dma_start(
                            wp_t[(pr, do)][:],
                            wp_d[pr * P:(pr + 1) * P, do * 512:(do + 1) * 512])

                copy_tick = [0]

                def emit_qk_group(pr, ch, w_t, dst, dve_only=False):
                    co = ch * 512
                    ps = psp.tile([P, 512], f32, tag="s", bufs=3,
                                  name="qk_ps")
                    for db in range(DB):
                        nc.tensor.matmul(
                            ps[:],
                            lhsT=w_t[db][:, pr * P:(pr + 1) * P],
                            rhs=xt_t[db][:, co:co + 512],
                            start=(db == 0), stop=(db == DB - 1))
                    copy_tick[0] += 1
                    if dve_only or copy_tick[0] % 2:
                        nc.vector.tensor_copy(dst[:, co:co + 512], ps[:])
                    else:
                        nc.scalar.copy(dst[:, co:co + 512], ps[:])

                def emit_v_group(ti):
                    ps = psp.tile([P, 512], f32, tag="s", bufs=3, name="v_ps")
                    for db in range(DB):
                        nc.tensor.matmul(
                            ps[:],
                            lhsT=xt_t[db][:, ti * P:(ti + 1) * P],
                            rhs=wv_t[db][:],
                            start=(db == 0), stop=(db == DB - 1))
                    nc.vector.tensor_copy(
                        v_t[ti][:].rearrange("p (h c) -> p h c",
                                             c=65)[:, :, 0:64],
                        ps[:].rearrange("p (h d) -> p h d", d=64))

                yc_tick = [0]

                def emit_proj(qh):
                    for tv in range(8):
                        ti = qh * 8 + tv
                        off = tv * P
                        for do in range(2):
                            yps = psp.tile([P, 512], f32, tag="s", bufs=3,
                                           name="y_ps")
                            for pr in range(NPAIR):
                                nc.tensor.matmul(
                                    yps[:],
                                    lhsT=ot_t[(pr, qh)][:, off:off + P],
                                    rhs=wp_t[(pr, do)][:],
                                    start=(pr == 0), stop=(pr == NPAIR - 1))
                            yt = y_pool.tile([P, 512], f32, tag="ysb",
                                             name="ysb")
                            yc_tick[0] += 1
                            if yc_tick[0] % 2:
                                nc.scalar.copy(yt[:], yps[:])
                            else:
                                nc.vector.tensor_copy(yt[:], yps[:])
                            nc.sync.dma_start(
                                y_d[ti * P:(ti + 1) * P,
                                    do * 512:(do + 1) * 512],
                                yt[:])

                # QK filler groups for the NEXT pair, spread across the
                # current pair's stages: {(pr, qb): [(w_t, target, ch), ...]}
                def qk_filler_plan():
                    plan = {}
                    for pr in range(3):
                        nxt = pr + 1
                        groups = []
                        for ch in range(4):
                            groups.append((wq_t, qt_t[nxt], nxt, ch))
                            groups.append((wk_t, kt_t[nxt], nxt, ch))
                        if pr == 0:
                            split = [(1, 3), (2, 3), (3, 2)]
                        else:
                            split = [(0, 2), (1, 2), (2, 2), (3, 2)]
                        i = 0
                        for qb, cnt in split:
                            plan[(pr, qb)] = groups[i:i + cnt]
                            i += cnt
                    return plan

                QK_FILL = qk_filler_plan()

                # deferred softmax normalization state
                pending_norm = [None]

                def emit_norm_flush():
                    if pending_norm[0] is None:
                        return
                    out_a, out_b, recb, dst_a, dst_b = pending_norm[0]
                    pending_norm[0] = None
                    rb_ps = psp.tile([P, 512], f32, tag="s", bufs=3,
                                     name="rb_ps")
                    nc.tensor.matmul(rb_ps[:], lhsT=sel33[0:1, :],
                                     rhs=recb[0:1, :], start=True,
                                     stop=False)
                    nc.tensor.matmul(rb_ps[:], lhsT=sel33[32:33, :],
                                     rhs=recb[32:33, :], start=False,
                                     stop=True)
                    rb = att_sb.tile([P, 512], f32, tag="rb", bufs=2,
                                     name="rb")
                    nc.scalar.copy(rb[:], rb_ps[:])
                    nc.vector.tensor_tensor(dst_a, out_a[0:64, :],
                                            rb[0:64, :], ALU.mult)
                    nc.vector.tensor_tensor(dst_b, out_b[0:64, :],
                                            rb[64:128, :], ALU.mult)

                def attention_stage(pr, qb):
                    ca = (2 * pr) * 65
                    cb = (2 * pr + 1) * 65
                    qo = qb * 512
                    out_a = psp.tile([65, 512], f32, tag="outa", bufs=1,
                                     name="out_a")
                    out_b = psp.tile([65, 512], f32, tag="outb", bufs=1,
                                     name="out_b")
                    e_tiles = {}
                    dve_set = dve_kjs(pr, qb)
                    fillers = {}
                    if pr == 0 and qb == 0:
                        for kj in range(1, 16):
                            fillers.setdefault(kj, []).append(
                                lambda kj=kj: emit_v_group(kj))
                    for gi, (w_t, dst, nxt, ch) in enumerate(
                            QK_FILL.get((pr, qb), [])):
                        kj_at = 1 + gi * 6
                        fillers.setdefault(kj_at, []).append(
                            lambda w_t=w_t, dst=dst, nxt=nxt, ch=ch:
                            emit_qk_group(nxt, ch, w_t, dst, dve_only=True))

                    def emit_scores(kj):
                        ko = kj * P
                        s_ab = psp.tile([P, 1024], f32, tag="s", bufs=3,
                                        name="s_ab")
                        nc.tensor.matmul(
                            s_ab[:, 0:512],
                            lhsT=kt_t[pr][0:64, ko:ko + P],
                            rhs=qt_t[pr][0:64, qo:qo + 512],
                            start=True, stop=True)
                        nc.tensor.matmul(
                            s_ab[:, 512:1024],
                            lhsT=kt_t[pr][64:128, ko:ko + P],
                            rhs=qt_t[pr][64:128, qo:qo + 512],
                            start=True, stop=True)
                        if no_exp:
                            e_ab = e_const
                        else:
                            e_ab = att_sb.tile([P, 1024], bf16, tag="e",
                                               bufs=8, name="e_ab")
                            if kj in dve_set:
                                nc.vector.tensor_scalar(
                                    e_ab[:].bitcast(i16), s_ab[:],
                                    float(EXPA), float(expb),
                                    ALU.mult, ALU.add)
                            else:
                                nc.scalar.activation(e_ab[:], s_ab[:],
                                                     AF.Exp, scale=SCALE)
                        e_tiles[kj] = e_ab

                    def emit_av(kj):
                        e_ab = e_tiles.pop(kj)
                        st = (kj == 0)
                        sp = (kj == 15)
                        nc.tensor.matmul(
                            out_a[:], lhsT=v_t[kj][:, ca:ca + 65],
                            rhs=e_ab[:, 0:512], start=st, stop=sp)
                        nc.tensor.matmul(
                            out_b[:], lhsT=v_t[kj][:, cb:cb + 65],
                            rhs=e_ab[:, 512:1024], start=st, stop=sp)

                    emit_scores(0)
                    # normalization of the previous stage, behind scores(0)
                    emit_norm_flush()
                    if pr == 3 and qb == 2:
                        emit_proj(0)
                    for kj in range(1, 16):
                        for f in fillers.get(kj, ()):
                            f()
                        emit_scores(kj)
                        emit_av(kj - 1)
                    emit_av(15)
                    # reciprocal of denominators (bf16 out, PSUM row 64)
                    recb = att_sb.tile([33, 512], bf16, tag="recb",
                                       bufs=2, name="recb")
                    with nc.allow_low_precision(
                            reason="softmax denom recip bf16"):
                        nc.vector.reciprocal(recb[0:1, :], out_a[64:65, :])
                        nc.vector.reciprocal(recb[32:33, :],
                                             out_b[64:65, :])
                    qco = (qb % 2) * 512
                    ot = ot_t[(pr, qb // 2)]
                    pending_norm[0] = (out_a, out_b, recb,
                                       ot[0:64, qco:qco + 512],
                                       ot[64:128, qco:qco + 512])

                # ---------------- schedule ----------------
                for ch in range(4):
                    emit_qk_group(0, ch, wq_t, qt_t[0])
                    emit_qk_group(0, ch, wk_t, kt_t[0])
                emit_v_group(0)
                for pr in range(NPAIR):
                    for qb in range(4):
                        attention_stage(pr, qb)
                emit_norm_flush()
                emit_proj(1)

    nc.compile()
    return nc


def _make_runner(nc):
    import jax
    from jax.sharding import Mesh, NamedSharding, PartitionSpec
    try:
        from jax import shard_map
        _shard_map = lambda f, mesh, in_specs, out_specs: shard_map(
            f, mesh=mesh, in_specs=in_specs, out_specs=out_specs,
            check_vma=False)
    except ImportError:
        from jax.experimental.shard_map import shard_map
        _shard_map = lambda f, mesh, in_specs, out_specs: shard_map(
            f, mesh=mesh, in_specs=in_specs, out_specs=out_specs,
            check_rep=False)
    import concourse.mybir as mybir
    from concourse.bass2jax import (_bass_exec_p, install_neuronx_cc_hook,
                                    partition_id_tensor)

    install_neuronx_cc_hook()

    partition_name = (nc.partition_id_tensor.name
                      if nc.partition_id_tensor else None)
    in_names, out_names, out_avals = [], [], []
    for alloc in nc.m.functions[0].allocations:
        if not isinstance(alloc, mybir.MemoryLocationSet):
            continue
        name = alloc.memorylocations[0].name
        if alloc.kind == "ExternalInput":
            if name != partition_name:
                in_names.append(name)
        elif alloc.kind == "ExternalOutput":
            out_names.append(name)
            out_avals.append(jax.core.ShapedArray(
                tuple(alloc.tensor_shape), mybir.dt.np(alloc.dtype)))

    n_params = len(in_names)
    all_in_names = list(in_names) + list(out_names)
    if partition_name is not None:
        all_in_names.append(partition_name)

    def _body(*args):
        operands = list(args)
        if partition_name is not None:
            operands.append(partition_id_tensor())
        outs = _bass_exec_p.bind(
            *operands,
            out_avals=tuple(out_avals),
            in_names=tuple(all_in_names),
            out_names=tuple(out_names),
            lowering_input_output_aliases=(),
            sim_require_finite=True,
            sim_require_nnan=True,
            nc=nc,
        )
        return tuple(outs)

    devices = jax.devices()[:N_CORES]
    mesh = Mesh(np.asarray(devices), ("core",))
    n_outs = len(out_avals)
    in_specs = (PartitionSpec("core"),) * (n_params + n_outs)
    out_specs = (PartitionSpec("core"),) * n_outs
    sharded = jax.jit(
        _shard_map(_body, mesh, in_specs, out_specs), keep_unused=True)
    sh = NamedSharding(mesh, PartitionSpec("core"))
    return {
        "fn": sharded,
        "in_names": in_names,
        "out_names": out_names,
        "out_avals": out_avals,
        "sharding": sh,
        "mesh": mesh,
    }


def _get_runner(repeat=1, **build_kwargs):
    key = ("runner", repeat, tuple(sorted(build_kwargs.items())))
    if key not in _CACHE:
        _CACHE[key] = _make_runner(_build_nc(repeat=repeat, **build_kwargs))
    return _CACHE[key]


def _shard_inputs(x, w_qkv, b_qkv, w_proj, b_proj):
    """Returns concatenated (along axis 0) per-core input arrays, in the
    order of the runner's in_names (xt, wq, wk, wv, wp)."""
    import ml_dtypes
    bf = ml_dtypes.bfloat16

    assert not np.any(np.asarray(b_qkv)), \
        "nonzero b_qkv not supported by this kernel build"

    per = {"xt": [], "wq": [], "wk": [], "wv": [], "wp": []}
    xtb = [np.ascontiguousarray(np.asarray(x)[b].T).astype(bf)
           for b in range(B)]
    w_qkv = np.asarray(w_qkv)
    w_proj = np.asarray(w_proj)
    wslices = {}
    for hh in range(2):
        wslices[("wq", hh)] = np.ascontiguousarray(
            w_qkv[:, 0 * D + hh * 512:0 * D + (hh + 1) * 512]).astype(bf)
        wslices[("wk", hh)] = np.ascontiguousarray(
            w_qkv[:, 1 * D + hh * 512:1 * D + (hh + 1) * 512]).astype(bf)
        wslices[("wv", hh)] = np.ascontiguousarray(
            w_qkv[:, 2 * D + hh * 512:2 * D + (hh + 1) * 512]).astype(bf)
        wslices[("wp", hh)] = np.ascontiguousarray(
            w_proj[hh * 512:(hh + 1) * 512, :]).astype(bf)
    for c in range(N_CORES):
        b, hh = divmod(c, 2)
        per["xt"].append(xtb[b])
        per["wq"].append(wslices[("wq", hh)])
        per["wk"].append(wslices[("wk", hh)])
        per["wv"].append(wslices[("wv", hh)])
        per["wp"].append(wslices[("wp", hh)])
    return {k: np.concatenate(v, axis=0) for k, v in per.items()}


def _run(runner, shards):
    import jax
    concat_in = [shards[name] for name in runner["in_names"]]
    concat_zeros = [
        np.zeros((N_CORES * av.shape[0],) + tuple(av.shape[1:]), av.dtype)
        for av in runner["out_avals"]
    ]
    outs = runner["fn"](*concat_in, *concat_zeros)
    jax.block_until_ready(outs)
    return {
        name: np.asarray(outs[i]).reshape(
            (N_CORES,) + tuple(runner["out_avals"][i].shape))
        for i, name in enumerate(runner["out_names"])
    }


def kernel(x, w_qkv, b_qkv, w_proj, b_proj):
    x = np.asarray(x)
    runner = _get_runner()
    shards = _shard_inputs(x, w_qkv, b_qkv, w_proj, b_proj)
    outs = _run(runner, shards)
    y = outs["y"]  # [8, S, D] fp32
    full = np.empty((B, S, D), np.float32)
    bp = np.asarray(b_proj, np.float32)
    for b in range(B):
        full[b] = y[2 * b] + y[2 * b + 1] + bp
    return full


# revision 29
# speedup vs baseline: 1.0188x; 1.0139x over previous
"""Multi-head attention block (QKV proj + softmax attention + out proj) on 8
Trainium2 NeuronCores.

Sharding: core c handles batch b = c//2 and head-half hh = c%2 (8 of the 16
heads).  Each core computes its QKV column slice, full attention for its 8
heads, and a partial output projection (contracting only its heads' dims).
Host sums the two partials per batch and adds b_proj.

Device layouts (all bf16 storage, fp32 PSUM accumulation):
  xT  [1024 d, 2048 tok]      (x[b] transposed on host)
  Q^T/K^T [512 col, 2048 tok] as 4 tiles [128, 2048]  (pair pr = heads 2pr,2pr+1)
  V   [2048 tok, 512 col]     as 16 tiles [128, 512]
  scores^T tiles [128 kj, 1024 qi] in PSUM -> exp -> SBUF bf16
  attn@V as out^T [128 (2 heads x 64 d), 1024 qi] accumulated in PSUM
  softmax denominators via ones-vector matmuls (col-tiled, partitions 0/32)

v2 performance structure:
  - exp is the per-stage bottleneck if left on ScalarE alone (~33.5M exps/core
    at 1 elem/cycle/lane).  Split per kj tile: most tiles use ScalarE AF.Exp,
    a statically chosen subset uses DVE Schraudolph (bits16 = s*EXPA + EXPB
    rounded to int16, bitcast bf16), sized so neither engine exceeds the PE
    work in that stage.
  - V-projection groups are interleaved into (pr0, qb0) attention; next
    pair's QK groups are interleaved into the current pair's stages.
  - Softmax normalization (recip/broadcast/scale) for stage i is emitted at
    the start of stage i+1 so the PE never waits on the DVE reciprocal.
"""

import math
import os

import numpy as np

B = 4
S = 2048
D = 1024
NUM_HEADS = 16
HD = 64
SCALE = HD**-0.5
# Schraudolph bf16 exp: bits16 = round(s*EXPA + EXPB); bitcast to bf16
EXPA = SCALE * 128.0 / math.log(2.0)
EXPB = 127.0 * 128.0 - 5.5
N_CORES = 8
P = 128
NPAIR = 4  # head pairs per core
DB = D // P  # 8 contraction blocks

_CACHE = {}


def _build_nc(repeat=1, no_exp=False, expb=EXPB, dve_split=True):
    import concourse.bacc as bacc
    import concourse.mybir as mybir
    import concourse.tile as tile

    bf16 = mybir.dt.bfloat16
    f32 = mybir.dt.float32
    i16 = mybir.dt.int16
    AF = mybir.ActivationFunctionType
    ALU = mybir.AluOpType

    nc = bacc.Bacc("TRN2", target_bir_lowering=False, debug=False,
                   num_devices=N_CORES)

    xt_d = nc.dram_tensor("xt", [D, S], bf16, kind="ExternalInput")
    wq_d = nc.dram_tensor("wq", [D, 512], bf16, kind="ExternalInput")
    wk_d = nc.dram_tensor("wk", [D, 512], bf16, kind="ExternalInput")
    wv_d = nc.dram_tensor("wv", [D, 512], bf16, kind="ExternalInput")
    wp_d = nc.dram_tensor("wp", [512, D], bf16, kind="ExternalInput")
    y_d = nc.dram_tensor("y", [S, D], f32, kind="ExternalOutput")

    # Number of kj tiles per stage handled by DVE Schraudolph, per (pr, qb).
    # Sized so ScalarE exp (+rb copy) stays under the stage's PE work.
    if dve_split:
        N_DVE = {
            (0, 0): 0, (0, 1): 1, (0, 2): 1, (0, 3): 3,
            (1, 0): 3, (1, 1): 3, (1, 2): 3, (1, 3): 3,
            (2, 0): 3, (2, 1): 3, (2, 2): 3, (2, 3): 3,
            (3, 0): 5, (3, 1): 0, (3, 2): 0, (3, 3): 0,
        }
    else:
        N_DVE = {(pr, qb): 0 for pr in range(4) for qb in range(4)}

    def dve_kjs(pr, qb):
        n = N_DVE[(pr, qb)]
        if n == 0:
            return frozenset()
        # spread over kj 1..11 — keep the last kj tiles on ScalarE so the
        # DVE queue is free for the end-of-stage reciprocals
        step = 11.0 / n
        return frozenset(1 + int(step * i + step / 2) for i in range(n))

    with tile.TileContext(nc) as tc:
        from contextlib import ExitStack, nullcontext

        with ExitStack() as ctx:
            const_pool = ctx.enter_context(tc.tile_pool(name="const", bufs=1))
            w_pool = ctx.enter_context(tc.tile_pool(name="w", bufs=1))
            wp_pool = ctx.enter_context(tc.tile_pool(name="wp", bufs=1))
            qt_pool = ctx.enter_context(tc.tile_pool(name="qt", bufs=1))
            kt_pool = ctx.enter_context(tc.tile_pool(name="kt", bufs=1))
            v_pool = ctx.enter_context(tc.tile_pool(name="v", bufs=1))
            ot_pool = ctx.enter_context(tc.tile_pool(name="ot", bufs=1))
            xt_pool = ctx.enter_context(tc.tile_pool(name="xt", bufs=1))
            att_sb = ctx.enter_context(tc.tile_pool(name="att_sb", bufs=2))
            y_pool = ctx.enter_context(tc.tile_pool(name="y", bufs=3))
            # PSUM banks: s 3x2 + outa 1 + outb 1 = 8
            psp = ctx.enter_context(tc.tile_pool(name="ps", bufs=1,
                                                 space="PSUM"))

            # head-block selectors for the rb broadcast (rows 0 / 32 so all
            # engine APs stay 32-partition-aligned):
            #   rb[0:64]   = recb[row 0]  (head a)
            #   rb[64:128] = recb[row 32] (head b)
            sel33 = const_pool.tile([33, P], bf16, tag="sel33", name="sel33")
            nc.vector.memset(sel33[:], 0.0)
            nc.vector.memset(sel33[0:1, 0:64], 1.0)
            nc.vector.memset(sel33[32:33, 64:128], 1.0)
            e_const = None
            if no_exp:
                e_const = const_pool.tile([P, 1024], bf16, tag="e_const",
                                          name="e_const")
                nc.vector.memset(e_const[:], 0.001)

            # weight tiles (loaded once, outside the repeat loop)
            wq_t = [w_pool.tile([P, 512], bf16, name=f"wq{i}")
                    for i in range(DB)]
            wk_t = [w_pool.tile([P, 512], bf16, name=f"wk{i}")
                    for i in range(DB)]
            wv_t = [w_pool.tile([P, 512], bf16, name=f"wv{i}")
                    for i in range(DB)]
            wp_t = {}
            for pr in range(NPAIR):
                for do in range(2):
                    wp_t[(pr, do)] = wp_pool.tile([P, 512], bf16,
                                                  name=f"wp{pr}_{do}")

            # ping-pong denominator-reciprocal tiles (rows 0 / 32 hold the
            # two heads' 1/den; other rows memset once and multiplied by
            # sel33's zero rows in the broadcast matmul)
            recb_t = [const_pool.tile([33, 512], bf16, name=f"recb{i}")
                      for i in range(2)]
            for i in range(2):
                nc.vector.memset(recb_t[i][:], 0.0)

            qt_t = [qt_pool.tile([P, S], bf16, name=f"qt{p}")
                    for p in range(NPAIR)]
            kt_t = [kt_pool.tile([P, S], bf16, name=f"kt{p}")
                    for p in range(NPAIR)]
            # V layout per token tile: 8 heads x [v_h (64) | ones (1)].
            # The ones column makes each AV matmul (M=65) also produce the
            # softmax denominator in PSUM row 64 — no separate den matmuls.
            v_t = [v_pool.tile([P, 8 * 65], bf16, name=f"v{i}")
                   for i in range(16)]
            for i in range(16):
                nc.vector.memset(
                    v_t[i][:].rearrange("p (h c) -> p h c", c=65)[:, :, 64:65],
                    1.0)
            ot_t = {}
            for pr in range(NPAIR):
                for qh in range(2):
                    ot_t[(pr, qh)] = ot_pool.tile([P, 1024], bf16,
                                                  name=f"ot{pr}_{qh}")

            # Weights are loaded once, outside the repeat loop (they don't
            # change across iterations); xt streams per iteration, chunked
            # in need-order so the first QK group starts after ~1 MB.
            for i in range(DB):
                nc.sync.dma_start(wq_t[i][:], wq_d[i * P:(i + 1) * P, :])
                nc.sync.dma_start(wk_t[i][:], wk_d[i * P:(i + 1) * P, :])
            for i in range(DB):
                nc.sync.dma_start(wv_t[i][:], wv_d[i * P:(i + 1) * P, :])
            for pr in range(NPAIR):
                for do in range(2):
                    nc.sync.dma_start(
                        wp_t[(pr, do)][:],
                        wp_d[pr * P:(pr + 1) * P, do * 512:(do + 1) * 512])

            loop_cm = (tc.For_i(0, repeat, 1) if repeat > 1 else nullcontext())
            with loop_cm:
                xt_t = [xt_pool.tile([P, S], bf16, tag=f"xt{i}",
                                     name=f"xt{i}") for i in range(DB)]
                for i in range(DB):
                    nc.sync.dma_start(xt_t[i][:, 0:512],
                                      xt_d[i * P:(i + 1) * P, 0:512])
                for ch in range(1, 4):
                    for i in range(DB):
                        nc.sync.dma_start(
                            xt_t[i][:, ch * 512:(ch + 1) * 512],
                            xt_d[i * P:(i + 1) * P, ch * 512:(ch + 1) * 512])

                copy_tick = [0]

                def emit_qk_group(pr, ch, w_t, dst, dve_only=False):
                    co = ch * 512
                    ps = psp.tile([P, 512], f32, tag="s", bufs=3,
                                  name="qk_ps")
                    for db in range(DB):
                        nc.tensor.matmul(
                            ps[:],
                            lhsT=w_t[db][:, pr * P:(pr + 1) * P],
                            rhs=xt_t[db][:, co:co + 512],
                            start=(db == 0), stop=(db == DB - 1))
                    copy_tick[0] += 1
                    if dve_only or copy_tick[0] % 2:
                        nc.vector.tensor_copy(dst[:, co:co + 512], ps[:])
                    else:
                        nc.scalar.copy(dst[:, co:co + 512], ps[:])

                def emit_v_group(ti):
                    ps = psp.tile([P, 512], f32, tag="s", bufs=3, name="v_ps")
                    for db in range(DB):
                        nc.tensor.matmul(
                            ps[:],
                            lhsT=xt_t[db][:, ti * P:(ti + 1) * P],
                            rhs=wv_t[db][:],
                            start=(db == 0), stop=(db == DB - 1))
                    nc.vector.tensor_copy(
                        v_t[ti][:].rearrange("p (h c) -> p h c",
                                             c=65)[:, :, 0:64],
                        ps[:].rearrange("p (h d) -> p h d", d=64))

                yc_tick = [0]

                def emit_proj(qh, half=None, dve_copies=False):
                    tvs = (range(8) if half is None
                           else range(4 * half, 4 * half + 4))
                    for tv in tvs:
                        ti = qh * 8 + tv
                        off = tv * P
                        for do in range(2):
                            yps = psp.tile([P, 512], f32, tag="s", bufs=3,
                                           name="y_ps")
                            for pr in range(NPAIR):
                                nc.tensor.matmul(
                                    yps[:],
                                    lhsT=ot_t[(pr, qh)][:, off:off + P],
                                    rhs=wp_t[(pr, do)][:],
                                    start=(pr == 0), stop=(pr == NPAIR - 1))
                            yt = y_pool.tile([P, 512], f32, tag="ysb",
                                             name="ysb")
                            yc_tick[0] += 1
                            if dve_copies or yc_tick[0] % 2 == 0:
                                nc.vector.tensor_copy(yt[:], yps[:])
                            else:
                                nc.scalar.copy(yt[:], yps[:])
                            nc.sync.dma_start(
                                y_d[ti * P:(ti + 1) * P,
                                    do * 512:(do + 1) * 512],
                                yt[:])

                # QK filler groups for the NEXT pair, spread across the
                # current pair's stages: {(pr, qb): [(w_t, target, ch), ...]}
                def qk_filler_plan():
                    plan = {}
                    for pr in range(3):
                        nxt = pr + 1
                        groups = []
                        for ch in range(4):
                            groups.append((wq_t, qt_t[nxt], nxt, ch))
                            groups.append((wk_t, kt_t[nxt], nxt, ch))
                        if pr == 0:
                            split = [(1, 3), (2, 3), (3, 2)]
                        else:
                            split = [(0, 2), (1, 2), (2, 2), (3, 2)]
                        i = 0
                        for qb, cnt in split:
                            plan[(pr, qb)] = groups[i:i + cnt]
                            i += cnt
                    return plan

                QK_FILL = qk_filler_plan()

                # deferred softmax normalization state
                pending_norm = [None]

                def emit_norm_flush():
                    if pending_norm[0] is None:
                        return
                    out_a, out_b, recb, dst_a, dst_b = pending_norm[0]
                    pending_norm[0] = None
                    rb_ps = psp.tile([P, 512], f32, tag="s", bufs=3,
                                     name="rb_ps")
                    nc.tensor.matmul(rb_ps[:], lhsT=sel33[0:33, :],
                                     rhs=recb[0:33, :], start=True,
                                     stop=True)
                    rb = att_sb.tile([P, 512], f32, tag="rb", bufs=2,
                                     name="rb")
                    nc.scalar.copy(rb[:], rb_ps[:])
                    nc.vector.tensor_tensor(dst_a, out_a[0:64, :],
                                            rb[0:64, :], ALU.mult)
                    nc.vector.tensor_tensor(dst_b, out_b[0:64, :],
                                            rb[64:128, :], ALU.mult)

                def attention_stage(pr, qb):
                    ca = (2 * pr) * 65
                    cb = (2 * pr + 1) * 65
                    qo = qb * 512
                    out_a = psp.tile([65, 512], f32, tag="outa", bufs=1,
                                     name="out_a")
                    out_b = psp.tile([65, 512], f32, tag="outb", bufs=1,
                                     name="out_b")
                    e_tiles = {}
                    dve_set = dve_kjs(pr, qb)
                    fillers = {}
                    if pr == 0 and qb == 0:
                        for kj in range(1, 16):
                            fillers.setdefault(kj, []).append(
                                lambda kj=kj: emit_v_group(kj))
                    for gi, (w_t, dst, nxt, ch) in enumerate(
                            QK_FILL.get((pr, qb), [])):
                        kj_at = 1 + gi * 6
                        fillers.setdefault(kj_at, []).append(
                            lambda w_t=w_t, dst=dst, nxt=nxt, ch=ch:
                            emit_qk_group(nxt, ch, w_t, dst, dve_only=True))

                    def emit_scores(kj):
                        ko = kj * P
                        s_ab = psp.tile([P, 1024], f32, tag="s", bufs=3,
                                        name="s_ab")
                        nc.tensor.matmul(
                            s_ab[:, 0:512],
                            lhsT=kt_t[pr][0:64, ko:ko + P],
                            rhs=qt_t[pr][0:64, qo:qo + 512],
                            start=True, stop=True)
                        nc.tensor.matmul(
                            s_ab[:, 512:1024],
                            lhsT=kt_t[pr][64:128, ko:ko + P],
                            rhs=qt_t[pr][64:128, qo:qo + 512],
                            start=True, stop=True)
                        if no_exp:
                            e_ab = e_const
                        else:
                            e_ab = att_sb.tile([P, 1024], bf16, tag="e",
                                               bufs=8, name="e_ab")
                            if kj in dve_set:
                                nc.vector.tensor_scalar(
                                    e_ab[:].bitcast(i16), s_ab[:],
                                    float(EXPA), float(expb),
                                    ALU.mult, ALU.add)
                            else:
                                nc.scalar.activation(e_ab[:], s_ab[:],
                                                     AF.Exp, scale=SCALE)
                        e_tiles[kj] = e_ab

                    def emit_av(kj):
                        e_ab = e_tiles.pop(kj)
                        st = (kj == 0)
                        sp = (kj == 15)
                        nc.tensor.matmul(
                            out_a[:], lhsT=v_t[kj][:, ca:ca + 65],
                            rhs=e_ab[:, 0:512], start=st, stop=sp)
                        nc.tensor.matmul(
                            out_b[:], lhsT=v_t[kj][:, cb:cb + 65],
                            rhs=e_ab[:, 512:1024], start=st, stop=sp)

                    emit_scores(0)
                    # normalization of the previous stage, behind scores(0)
                    emit_norm_flush()
                    # output projection halves, pipelined into pr3 stages as
                    # soon as the ot columns they contract become available
                    if pr == 3 and qb == 1:
                        emit_proj(0, half=0, dve_copies=True)
                    elif pr == 3 and qb == 2:
                        emit_proj(0, half=1, dve_copies=True)
                    elif pr == 3 and qb == 3:
                        emit_proj(1, half=0, dve_copies=True)
                    for kj in range(1, 16):
                        for f in fillers.get(kj, ()):
                            f()
                        emit_scores(kj)
                        emit_av(kj - 1)
                    emit_av(15)
                    # reciprocal of denominators (bf16 out, PSUM row 64)
                    recb = recb_t[(4 * pr + qb) % 2]
                    with nc.allow_low_precision(
                            reason="softmax denom recip bf16"):
                        nc.vector.reciprocal(recb[0:1, :], out_a[64:65, :])
                        nc.vector.reciprocal(recb[32:33, :],
                                             out_b[64:65, :])
                    qco = (qb % 2) * 512
                    ot = ot_t[(pr, qb // 2)]
                    pending_norm[0] = (out_a, out_b, recb,
                                       ot[0:64, qco:qco + 512],
                                       ot[64:128, qco:qco + 512])

                # ---------------- schedule ----------------
                for ch in range(4):
                    emit_qk_group(0, ch, wq_t, qt_t[0])
                    emit_qk_group(0, ch, wk_t, kt_t[0])
                emit_v_group(0)
                for pr in range(NPAIR):
                    for qb in range(4):
                        attention_stage(pr, qb)
                emit_norm_flush()
                emit_proj(1, half=1)

    nc.compile()
    return nc


def _make_runner(nc):
    import jax
    from jax.sharding import Mesh, NamedSharding, PartitionSpec
    try:
        from jax import shard_map
        _shard_map = lambda f, mesh, in_specs, out_specs: shard_map(
            f, mesh=mesh, in_specs=in_specs, out_specs=out_specs,
            check_vma=False)
    except ImportError:
        from jax.experimental.shard_map import shard_map
        _shard_map = lambda f, mesh, in_specs, out_specs: shard_map(
            f, mesh=mesh, in_specs=in_specs, out_specs=out_specs,
            check_rep=False)
    import concourse.mybir as mybir
    from concourse.bass2jax import (_bass_exec_p, install_neuronx_cc_hook,
                                    partition_id_tensor)

    install_neuronx_cc_hook()

    partition_name = (nc.partition_id_tensor.name
                      if nc.partition_id_tensor else None)
    in_names, out_names, out_avals = [], [], []
    for alloc in nc.m.functions[0].allocations:
        if not isinstance(alloc, mybir.MemoryLocationSet):
            continue
        name = alloc.memorylocations[0].name
        if alloc.kind == "ExternalInput":
            if name != partition_name:
                in_names.append(name)
        elif alloc.kind == "ExternalOutput":
            out_names.append(name)
            out_avals.append(jax.core.ShapedArray(
                tuple(alloc.tensor_shape), mybir.dt.np(alloc.dtype)))

    n_params = len(in_names)
    all_in_names = list(in_names) + list(out_names)
    if partition_name is not None:
        all_in_names.append(partition_name)

    def _body(*args):
        operands = list(args)
        if partition_name is not None:
            operands.append(partition_id_tensor())
        outs = _bass_exec_p.bind(
            *operands,
            out_avals=tuple(out_avals),
            in_names=tuple(all_in_names),
            out_names=tuple(out_names),
            lowering_input_output_aliases=(),
            sim_require_finite=True,
            sim_require_nnan=True,
            nc=nc,
        )
        return tuple(outs)

    devices = jax.devices()[:N_CORES]
    mesh = Mesh(np.asarray(devices), ("core",))
    n_outs = len(out_avals)
    in_specs = (PartitionSpec("core"),) * (n_params + n_outs)
    out_specs = (PartitionSpec("core"),) * n_outs
    sharded = jax.jit(
        _shard_map(_body, mesh, in_specs, out_specs), keep_unused=True)
    sh = NamedSharding(mesh, PartitionSpec("core"))
    return {
        "fn": sharded,
        "in_names": in_names,
        "out_names": out_names,
        "out_avals": out_avals,
        "sharding": sh,
        "mesh": mesh,
    }


def _get_runner(repeat=1, **build_kwargs):
    key = ("runner", repeat, tuple(sorted(build_kwargs.items())))
    if key not in _CACHE:
        _CACHE[key] = _make_runner(_build_nc(repeat=repeat, **build_kwargs))
    return _CACHE[key]


def _shard_inputs(x, w_qkv, b_qkv, w_proj, b_proj):
    """Returns concatenated (along axis 0) per-core input arrays, in the
    order of the runner's in_names (xt, wq, wk, wv, wp)."""
    import ml_dtypes
    bf = ml_dtypes.bfloat16

    assert not np.any(np.asarray(b_qkv)), \
        "nonzero b_qkv not supported by this kernel build"

    per = {"xt": [], "wq": [], "wk": [], "wv": [], "wp": []}
    xtb = [np.ascontiguousarray(np.asarray(x)[b].T).astype(bf)
           for b in range(B)]
    w_qkv = np.asarray(w_qkv)
    w_proj = np.asarray(w_proj)
    wslices = {}
    for hh in range(2):
        wslices[("wq", hh)] = np.ascontiguousarray(
            w_qkv[:, 0 * D + hh * 512:0 * D + (hh + 1) * 512]).astype(bf)
        wslices[("wk", hh)] = np.ascontiguousarray(
            w_qkv[:, 1 * D + hh * 512:1 * D + (hh + 1) * 512]).astype(bf)
        wslices[("wv", hh)] = np.ascontiguousarray(
            w_qkv[:, 2 * D + hh * 512:2 * D + (hh + 1) * 512]).astype(bf)
        wslices[("wp", hh)] = np.ascontiguousarray(
            w_proj[hh * 512:(hh + 1) * 512, :]).astype(bf)
    for c in range(N_CORES):
        b, hh = divmod(c, 2)
        per["xt"].append(xtb[b])
        per["wq"].append(wslices[("wq", hh)])
        per["wk"].append(wslices[("wk", hh)])
        per["wv"].append(wslices[("wv", hh)])
        per["wp"].append(wslices[("wp", hh)])
    return {k: np.concatenate(v, axis=0) for k, v in per.items()}


def _run(runner, shards):
    import jax
    concat_in = [shards[name] for name in runner["in_names"]]
    concat_zeros = [
        np.zeros((N_CORES * av.shape[0],) + tuple(av.shape[1:]), av.dtype)
        for av in runner["out_avals"]
    ]
    outs = runner["fn"](*concat_in, *concat_zeros)
    jax.block_until_ready(outs)
    return {
        name: np.asarray(outs[i]).reshape(
            (N_CORES,) + tuple(runner["out_avals"][i].shape))
        for i, name in enumerate(runner["out_names"])
    }


def kernel(x, w_qkv, b_qkv, w_proj, b_proj):
    x = np.asarray(x)
    runner = _get_runner()
    shards = _shard_inputs(x, w_qkv, b_qkv, w_proj, b_proj)
    outs = _run(runner, shards)
    y = outs["y"]  # [8, S, D] fp32
    full = np.empty((B, S, D), np.float32)
    bp = np.asarray(b_proj, np.float32)
    for b in range(B):
        full[b] = y[2 * b] + y[2 * b + 1] + bp
    return full


# revision 30
# speedup vs baseline: 1.0440x; 1.0247x over previous
"""Multi-head attention block (QKV proj + softmax attention + out proj) on 8
Trainium2 NeuronCores.

Sharding: core c handles batch b = c//2 and head-half hh = c%2 (8 of the 16
heads).  Each core computes its QKV column slice, full attention for its 8
heads, and a partial output projection (contracting only its heads' dims).
Host sums the two partials per batch and adds b_proj.

Device layouts (all bf16 storage, fp32 PSUM accumulation):
  xT  [1024 d, 2048 tok]      (x[b] transposed on host)
  Q^T/K^T [512 col, 2048 tok] as 4 tiles [128, 2048]  (pair pr = heads 2pr,2pr+1)
  V   [2048 tok, 8 x (64 v | 1 ones)] as 16 tiles [128, 520]
  scores^T tiles [128 kj, 1024 qi] in PSUM -> exp -> SBUF bf16
  attn@V as two [65, 512] PSUM accumulators per stage (M=65: the ones column
  of V makes row 64 the softmax denominator -- no separate den matmuls)

Performance structure:
  - exp would be the per-stage bottleneck left on ScalarE alone (~33.5M
    exps/core at 1 elem/cycle/lane).  Split per kj tile: most tiles use
    ScalarE AF.Exp, a statically chosen subset uses DVE Schraudolph
    (bits16 = s*EXPA + EXPB rounded to int16, bitcast bf16), sized so
    neither engine exceeds the PE work in that stage (34/256 tiles on DVE;
    measured rel err ~9e-3 vs 6.5e-3 all-ScalarE, gate 2e-2).
  - V-projection groups are interleaved into (pr0, qb0) attention; next
    pair's QK groups are interleaved into the current pair's stages; the
    output projection is emitted in quarter chunks inside the pr3 stages.
  - Softmax normalization (recip / sel33 broadcast-matmul / scale) for
    stage i is emitted at the start of stage i+1 so the PE never waits on
    the DVE reciprocal; weights are DMA'd once outside the repeat loop.
"""

import math
import os

import numpy as np

B = 4
S = 2048
D = 1024
NUM_HEADS = 16
HD = 64
SCALE = HD**-0.5
# Schraudolph bf16 exp: bits16 = round(s*EXPA + EXPB); bitcast to bf16
EXPA = SCALE * 128.0 / math.log(2.0)
EXPB = 127.0 * 128.0 - 5.5
N_CORES = 8
P = 128
NPAIR = 4  # head pairs per core
DB = D // P  # 8 contraction blocks

_CACHE = {}


def _build_nc(repeat=1, no_exp=False, expb=EXPB, dve_split=True):
    import concourse.bacc as bacc
    import concourse.mybir as mybir
    import concourse.tile as tile

    bf16 = mybir.dt.bfloat16
    f32 = mybir.dt.float32
    i16 = mybir.dt.int16
    AF = mybir.ActivationFunctionType
    ALU = mybir.AluOpType

    nc = bacc.Bacc("TRN2", target_bir_lowering=False, debug=False,
                   num_devices=N_CORES)

    xt_d = nc.dram_tensor("xt", [D, S], bf16, kind="ExternalInput")
    wq_d = nc.dram_tensor("wq", [D, 512], bf16, kind="ExternalInput")
    wk_d = nc.dram_tensor("wk", [D, 512], bf16, kind="ExternalInput")
    wv_d = nc.dram_tensor("wv", [D, 512], bf16, kind="ExternalInput")
    wp_d = nc.dram_tensor("wp", [512, D], bf16, kind="ExternalInput")
    y_d = nc.dram_tensor("y", [S, D], f32, kind="ExternalOutput")

    # Number of kj tiles per stage handled by DVE Schraudolph, per (pr, qb).
    # Sized so ScalarE exp (+rb copy) stays under the stage's PE work.
    if dve_split:
        N_DVE = {
            (0, 0): 0, (0, 1): 1, (0, 2): 1, (0, 3): 3,
            (1, 0): 3, (1, 1): 3, (1, 2): 3, (1, 3): 3,
            (2, 0): 3, (2, 1): 3, (2, 2): 3, (2, 3): 3,
            (3, 0): 5, (3, 1): 0, (3, 2): 0, (3, 3): 0,
        }
    else:
        N_DVE = {(pr, qb): 0 for pr in range(4) for qb in range(4)}

    def dve_kjs(pr, qb):
        n = N_DVE[(pr, qb)]
        if n == 0:
            return frozenset()
        # spread over kj 1..11 — keep the last kj tiles on ScalarE so the
        # DVE queue is free for the end-of-stage reciprocals
        step = 11.0 / n
        return frozenset(1 + int(step * i + step / 2) for i in range(n))

    with tile.TileContext(nc) as tc:
        from contextlib import ExitStack, nullcontext

        with ExitStack() as ctx:
            const_pool = ctx.enter_context(tc.tile_pool(name="const", bufs=1))
            w_pool = ctx.enter_context(tc.tile_pool(name="w", bufs=1))
            wp_pool = ctx.enter_context(tc.tile_pool(name="wp", bufs=1))
            qt_pool = ctx.enter_context(tc.tile_pool(name="qt", bufs=1))
            kt_pool = ctx.enter_context(tc.tile_pool(name="kt", bufs=1))
            v_pool = ctx.enter_context(tc.tile_pool(name="v", bufs=1))
            ot_pool = ctx.enter_context(tc.tile_pool(name="ot", bufs=1))
            xt_pool = ctx.enter_context(tc.tile_pool(name="xt", bufs=1))
            att_sb = ctx.enter_context(tc.tile_pool(name="att_sb", bufs=2))
            y_pool = ctx.enter_context(tc.tile_pool(name="y", bufs=3))
            # PSUM banks: s 3x2 + outa 1 + outb 1 = 8
            psp = ctx.enter_context(tc.tile_pool(name="ps", bufs=1,
                                                 space="PSUM"))

            # head-block selectors for the rb broadcast (rows 0 / 32 so all
            # engine APs stay 32-partition-aligned):
            #   rb[0:64]   = recb[row 0]  (head a)
            #   rb[64:128] = recb[row 32] (head b)
            sel33 = const_pool.tile([33, P], bf16, tag="sel33", name="sel33")
            nc.vector.memset(sel33[:], 0.0)
            nc.vector.memset(sel33[0:1, 0:64], 1.0)
            nc.vector.memset(sel33[32:33, 64:128], 1.0)
            e_const = None
            if no_exp:
                e_const = const_pool.tile([P, 1024], bf16, tag="e_const",
                                          name="e_const")
                nc.vector.memset(e_const[:], 0.001)

            # weight tiles (loaded once, outside the repeat loop)
            wq_t = [w_pool.tile([P, 512], bf16, name=f"wq{i}")
                    for i in range(DB)]
            wk_t = [w_pool.tile([P, 512], bf16, name=f"wk{i}")
                    for i in range(DB)]
            wv_t = [w_pool.tile([P, 512], bf16, name=f"wv{i}")
                    for i in range(DB)]
            wp_t = {}
            for pr in range(NPAIR):
                for do in range(2):
                    wp_t[(pr, do)] = wp_pool.tile([P, 512], bf16,
                                                  name=f"wp{pr}_{do}")

            # ping-pong denominator-reciprocal tiles (rows 0 / 32 hold the
            # two heads' 1/den; other rows memset once and multiplied by
            # sel33's zero rows in the broadcast matmul)
            recb_t = [const_pool.tile([33, 512], bf16, name=f"recb{i}")
                      for i in range(2)]
            for i in range(2):
                nc.vector.memset(recb_t[i][:], 0.0)

            qt_t = [qt_pool.tile([P, S], bf16, name=f"qt{p}")
                    for p in range(NPAIR)]
            kt_t = [kt_pool.tile([P, S], bf16, name=f"kt{p}")
                    for p in range(NPAIR)]
            # V layout per token tile: 8 heads x [v_h (64) | ones (1)].
            # The ones column makes each AV matmul (M=65) also produce the
            # softmax denominator in PSUM row 64 — no separate den matmuls.
            v_t = [v_pool.tile([P, 8 * 65], bf16, name=f"v{i}")
                   for i in range(16)]
            for i in range(16):
                nc.vector.memset(
                    v_t[i][:].rearrange("p (h c) -> p h c", c=65)[:, :, 64:65],
                    1.0)
            ot_t = {}
            for pr in range(NPAIR):
                for qh in range(2):
                    ot_t[(pr, qh)] = ot_pool.tile([P, 1024], bf16,
                                                  name=f"ot{pr}_{qh}")

            # Weights are loaded once, outside the repeat loop (they don't
            # change across iterations); xt streams per iteration, chunked
            # in need-order so the first QK group starts after ~1 MB.
            for i in range(DB):
                nc.sync.dma_start(wq_t[i][:], wq_d[i * P:(i + 1) * P, :])
                nc.sync.dma_start(wk_t[i][:], wk_d[i * P:(i + 1) * P, :])
            for i in range(DB):
                nc.sync.dma_start(wv_t[i][:], wv_d[i * P:(i + 1) * P, :])
            for pr in range(NPAIR):
                for do in range(2):
                    nc.sync.dma_start(
                        wp_t[(pr, do)][:],
                        wp_d[pr * P:(pr + 1) * P, do * 512:(do + 1) * 512])

            loop_cm = (tc.For_i(0, repeat, 1) if repeat > 1 else nullcontext())
            with loop_cm:
                xt_t = [xt_pool.tile([P, S], bf16, tag=f"xt{i}",
                                     name=f"xt{i}") for i in range(DB)]
                for i in range(DB):
                    nc.sync.dma_start(xt_t[i][:, 0:512],
                                      xt_d[i * P:(i + 1) * P, 0:512])
                for ch in range(1, 4):
                    for i in range(DB):
                        nc.sync.dma_start(
                            xt_t[i][:, ch * 512:(ch + 1) * 512],
                            xt_d[i * P:(i + 1) * P, ch * 512:(ch + 1) * 512])

                copy_tick = [0]

                def emit_qk_group(pr, ch, w_t, dst, dve_only=False):
                    co = ch * 512
                    ps = psp.tile([P, 512], f32, tag="s", bufs=3,
                                  name="qk_ps")
                    for db in range(DB):
                        nc.tensor.matmul(
                            ps[:],
                            lhsT=w_t[db][:, pr * P:(pr + 1) * P],
                            rhs=xt_t[db][:, co:co + 512],
                            start=(db == 0), stop=(db == DB - 1))
                    copy_tick[0] += 1
                    if dve_only or copy_tick[0] % 2:
                        nc.vector.tensor_copy(dst[:, co:co + 512], ps[:])
                    else:
                        nc.scalar.copy(dst[:, co:co + 512], ps[:])

                def emit_v_group(ti):
                    ps = psp.tile([P, 512], f32, tag="s", bufs=3, name="v_ps")
                    for db in range(DB):
                        nc.tensor.matmul(
                            ps[:],
                            lhsT=xt_t[db][:, ti * P:(ti + 1) * P],
                            rhs=wv_t[db][:],
                            start=(db == 0), stop=(db == DB - 1))
                    nc.vector.tensor_copy(
                        v_t[ti][:].rearrange("p (h c) -> p h c",
                                             c=65)[:, :, 0:64],
                        ps[:].rearrange("p (h d) -> p h d", d=64))

                yc_tick = [0]

                def emit_proj(qh, half=None, dve_copies=False):
                    tvs = (range(8) if half is None
                           else range(4 * half, 4 * half + 4))
                    for tv in tvs:
                        ti = qh * 8 + tv
                        off = tv * P
                        for do in range(2):
                            yps = psp.tile([P, 512], f32, tag="s", bufs=3,
                                           name="y_ps")
                            for pr in range(NPAIR):
                                nc.tensor.matmul(
                                    yps[:],
                                    lhsT=ot_t[(pr, qh)][:, off:off + P],
                                    rhs=wp_t[(pr, do)][:],
                                    start=(pr == 0), stop=(pr == NPAIR - 1))
                            yt = y_pool.tile([P, 512], f32, tag="ysb",
                                             name="ysb")
                            yc_tick[0] += 1
                            if dve_copies or yc_tick[0] % 2 == 0:
                                nc.vector.tensor_copy(yt[:], yps[:])
                            else:
                                nc.scalar.copy(yt[:], yps[:])
                            nc.sync.dma_start(
                                y_d[ti * P:(ti + 1) * P,
                                    do * 512:(do + 1) * 512],
                                yt[:])

                # QK filler groups for the NEXT pair, spread across the
                # current pair's stages: {(pr, qb): [(w_t, target, ch), ...]}
                def qk_filler_plan():
                    plan = {}
                    for pr in range(3):
                        nxt = pr + 1
                        groups = []
                        for ch in range(4):
                            groups.append((wq_t, qt_t[nxt], nxt, ch))
                            groups.append((wk_t, kt_t[nxt], nxt, ch))
                        if pr == 0:
                            split = [(1, 3), (2, 3), (3, 2)]
                        else:
                            split = [(0, 2), (1, 2), (2, 2), (3, 2)]
                        i = 0
                        for qb, cnt in split:
                            plan[(pr, qb)] = groups[i:i + cnt]
                            i += cnt
                    return plan

                QK_FILL = qk_filler_plan()

                # deferred softmax normalization state
                pending_norm = [None]

                def emit_norm_flush():
                    if pending_norm[0] is None:
                        return
                    out_a, out_b, recb, dst_a, dst_b = pending_norm[0]
                    pending_norm[0] = None
                    rb_ps = psp.tile([P, 512], f32, tag="s", bufs=3,
                                     name="rb_ps")
                    nc.tensor.matmul(rb_ps[:], lhsT=sel33[0:33, :],
                                     rhs=recb[0:33, :], start=True,
                                     stop=True)
                    rb = att_sb.tile([P, 512], f32, tag="rb", bufs=2,
                                     name="rb")
                    nc.scalar.copy(rb[:], rb_ps[:])
                    nc.vector.tensor_tensor(dst_a, out_a[0:64, :],
                                            rb[0:64, :], ALU.mult)
                    nc.vector.tensor_tensor(dst_b, out_b[0:64, :],
                                            rb[64:128, :], ALU.mult)

                def attention_stage(pr, qb):
                    ca = (2 * pr) * 65
                    cb = (2 * pr + 1) * 65
                    qo = qb * 512
                    out_a = psp.tile([65, 512], f32, tag="outa", bufs=1,
                                     name="out_a")
                    out_b = psp.tile([65, 512], f32, tag="outb", bufs=1,
                                     name="out_b")
                    e_tiles = {}
                    dve_set = dve_kjs(pr, qb)
                    fillers = {}
                    if pr == 0 and qb == 0:
                        for kj in range(1, 16):
                            fillers.setdefault(kj, []).append(
                                lambda kj=kj: emit_v_group(kj))
                    for gi, (w_t, dst, nxt, ch) in enumerate(
                            QK_FILL.get((pr, qb), [])):
                        kj_at = 1 + gi * 6
                        fillers.setdefault(kj_at, []).append(
                            lambda w_t=w_t, dst=dst, nxt=nxt, ch=ch:
                            emit_qk_group(nxt, ch, w_t, dst, dve_only=True))

                    def emit_scores(kj):
                        ko = kj * P
                        s_ab = psp.tile([P, 1024], f32, tag="s", bufs=3,
                                        name="s_ab")
                        nc.tensor.matmul(
                            s_ab[:, 0:512],
                            lhsT=kt_t[pr][0:64, ko:ko + P],
                            rhs=qt_t[pr][0:64, qo:qo + 512],
                            start=True, stop=True)
                        nc.tensor.matmul(
                            s_ab[:, 512:1024],
                            lhsT=kt_t[pr][64:128, ko:ko + P],
                            rhs=qt_t[pr][64:128, qo:qo + 512],
                            start=True, stop=True)
                        if no_exp:
                            e_ab = e_const
                        else:
                            e_ab = att_sb.tile([P, 1024], bf16, tag="e",
                                               bufs=8, name="e_ab")
                            if kj in dve_set:
                                nc.vector.tensor_scalar(
                                    e_ab[:].bitcast(i16), s_ab[:],
                                    float(EXPA), float(expb),
                                    ALU.mult, ALU.add)
                            else:
                                nc.scalar.activation(e_ab[:], s_ab[:],
                                                     AF.Exp, scale=SCALE)
                        e_tiles[kj] = e_ab

                    def emit_av(kj):
                        e_ab = e_tiles.pop(kj)
                        st = (kj == 0)
                        sp = (kj == 15)
                        nc.tensor.matmul(
                            out_a[:], lhsT=v_t[kj][:, ca:ca + 65],
                            rhs=e_ab[:, 0:512], start=st, stop=sp)
                        nc.tensor.matmul(
                            out_b[:], lhsT=v_t[kj][:, cb:cb + 65],
                            rhs=e_ab[:, 512:1024], start=st, stop=sp)

                    emit_scores(0)
                    # normalization of the previous stage, behind scores(0)
                    emit_norm_flush()
                    # output projection halves, pipelined into pr3 stages as
                    # soon as the ot columns they contract become available
                    if pr == 3 and qb == 1:
                        emit_proj(0, half=0, dve_copies=True)
                    elif pr == 3 and qb == 2:
                        emit_proj(0, half=1, dve_copies=True)
                    elif pr == 3 and qb == 3:
                        emit_proj(1, half=0, dve_copies=True)
                    for kj in range(1, 16):
                        for f in fillers.get(kj, ()):
                            f()
                        emit_scores(kj)
                        emit_av(kj - 1)
                    emit_av(15)
                    # reciprocal of denominators (bf16 out, PSUM row 64)
                    recb = recb_t[(4 * pr + qb) % 2]
                    with nc.allow_low_precision(
                            reason="softmax denom recip bf16"):
                        nc.vector.reciprocal(recb[0:1, :], out_a[64:65, :])
                        nc.vector.reciprocal(recb[32:33, :],
                                             out_b[64:65, :])
                    qco = (qb % 2) * 512
                    ot = ot_t[(pr, qb // 2)]
                    pending_norm[0] = (out_a, out_b, recb,
                                       ot[0:64, qco:qco + 512],
                                       ot[64:128, qco:qco + 512])

                # ---------------- schedule ----------------
                for ch in range(4):
                    emit_qk_group(0, ch, wq_t, qt_t[0])
                    emit_qk_group(0, ch, wk_t, kt_t[0])
                emit_v_group(0)
                for pr in range(NPAIR):
                    for qb in range(4):
                        attention_stage(pr, qb)
                emit_norm_flush()
                emit_proj(1, half=1)

    nc.compile()
    return nc


def _make_runner(nc):
    import jax
    from jax.sharding import Mesh, NamedSharding, PartitionSpec
    try:
        from jax import shard_map
        _shard_map = lambda f, mesh, in_specs, out_specs: shard_map(
            f, mesh=mesh, in_specs=in_specs, out_specs=out_specs,
            check_vma=False)
    except ImportError:
        from jax.experimental.shard_map import shard_map
        _shard_map = lambda f, mesh, in_specs, out_specs: shard_map(
            f, mesh=mesh, in_specs=in_specs, out_specs=out_specs,
            check_rep=False)
    import concourse.mybir as mybir
    from concourse.bass2jax import (_bass_exec_p, install_neuronx_cc_hook,
                                    partition_id_tensor)

    install_neuronx_cc_hook()

    partition_name = (nc.partition_id_tensor.name
                      if nc.partition_id_tensor else None)
    in_names, out_names, out_avals = [], [], []
    for alloc in nc.m.functions[0].allocations:
        if not isinstance(alloc, mybir.MemoryLocationSet):
            continue
        name = alloc.memorylocations[0].name
        if alloc.kind == "ExternalInput":
            if name != partition_name:
                in_names.append(name)
        elif alloc.kind == "ExternalOutput":
            out_names.append(name)
            out_avals.append(jax.core.ShapedArray(
                tuple(alloc.tensor_shape), mybir.dt.np(alloc.dtype)))

    n_params = len(in_names)
    all_in_names = list(in_names) + list(out_names)
    if partition_name is not None:
        all_in_names.append(partition_name)

    def _body(*args):
        operands = list(args)
        if partition_name is not None:
            operands.append(partition_id_tensor())
        outs = _bass_exec_p.bind(
            *operands,
            out_avals=tuple(out_avals),
            in_names=tuple(all_in_names),
            out_names=tuple(out_names),
            lowering_input_output_aliases=(),
            sim_require_finite=True,
            sim_require_nnan=True,
            nc=nc,
        )
        return tuple(outs)

    devices = jax.devices()[:N_CORES]
    mesh = Mesh(np.asarray(devices), ("core",))
    n_outs = len(out_avals)
    in_specs = (PartitionSpec("core"),) * (n_params + n_outs)
    out_specs = (PartitionSpec("core"),) * n_outs
    sharded = jax.jit(
        _shard_map(_body, mesh, in_specs, out_specs), keep_unused=True)
    sh = NamedSharding(mesh, PartitionSpec("core"))
    return {
        "fn": sharded,
        "in_names": in_names,
        "out_names": out_names,
        "out_avals": out_avals,
        "sharding": sh,
        "mesh": mesh,
    }


def _get_runner(repeat=1, **build_kwargs):
    key = ("runner", repeat, tuple(sorted(build_kwargs.items())))
    if key not in _CACHE:
        _CACHE[key] = _make_runner(_build_nc(repeat=repeat, **build_kwargs))
    return _CACHE[key]


def _shard_inputs(x, w_qkv, b_qkv, w_proj, b_proj):
    """Returns concatenated (along axis 0) per-core input arrays, in the
    order of the runner's in_names (xt, wq, wk, wv, wp)."""
    import ml_dtypes
    bf = ml_dtypes.bfloat16

    assert not np.any(np.asarray(b_qkv)), \
        "nonzero b_qkv not supported by this kernel build"

    per = {"xt": [], "wq": [], "wk": [], "wv": [], "wp": []}
    xtb = [np.ascontiguousarray(np.asarray(x)[b].T).astype(bf)
           for b in range(B)]
    w_qkv = np.asarray(w_qkv)
    w_proj = np.asarray(w_proj)
    wslices = {}
    for hh in range(2):
        wslices[("wq", hh)] = np.ascontiguousarray(
            w_qkv[:, 0 * D + hh * 512:0 * D + (hh + 1) * 512]).astype(bf)
        wslices[("wk", hh)] = np.ascontiguousarray(
            w_qkv[:, 1 * D + hh * 512:1 * D + (hh + 1) * 512]).astype(bf)
        wslices[("wv", hh)] = np.ascontiguousarray(
            w_qkv[:, 2 * D + hh * 512:2 * D + (hh + 1) * 512]).astype(bf)
        wslices[("wp", hh)] = np.ascontiguousarray(
            w_proj[hh * 512:(hh + 1) * 512, :]).astype(bf)
    for c in range(N_CORES):
        b, hh = divmod(c, 2)
        per["xt"].append(xtb[b])
        per["wq"].append(wslices[("wq", hh)])
        per["wk"].append(wslices[("wk", hh)])
        per["wv"].append(wslices[("wv", hh)])
        per["wp"].append(wslices[("wp", hh)])
    return {k: np.concatenate(v, axis=0) for k, v in per.items()}


def _run(runner, shards):
    import jax
    concat_in = [shards[name] for name in runner["in_names"]]
    concat_zeros = [
        np.zeros((N_CORES * av.shape[0],) + tuple(av.shape[1:]), av.dtype)
        for av in runner["out_avals"]
    ]
    outs = runner["fn"](*concat_in, *concat_zeros)
    jax.block_until_ready(outs)
    return {
        name: np.asarray(outs[i]).reshape(
            (N_CORES,) + tuple(runner["out_avals"][i].shape))
        for i, name in enumerate(runner["out_names"])
    }


def kernel(x, w_qkv, b_qkv, w_proj, b_proj):
    x = np.asarray(x)
    runner = _get_runner()
    shards = _shard_inputs(x, w_qkv, b_qkv, w_proj, b_proj)
    outs = _run(runner, shards)
    y = outs["y"]  # [8, S, D] fp32
    full = np.empty((B, S, D), np.float32)
    bp = np.asarray(b_proj, np.float32)
    for b in range(B):
        full[b] = y[2 * b] + y[2 * b + 1] + bp
    return full
